# revision 17
# baseline (speedup 1.0000x reference)
"""CapsuleLayer (dynamic routing) Trainium2 kernel.

x: [128, 2048, 8] f32, W: [2048, 32, 8, 16] f32 -> v: [128, 32, 16] f32

Sharding: batch B=128 split across 8 cores (16 each), W replicated.
Per core, per routing pass, u_hat tiles ([128,512] = 16 caps x 8 batch
x 512 (o,d)) are recomputed on the PE via a block-diagonal-x matmul and
consumed on-chip; u_hat never touches HBM.
"""

from contextlib import ExitStack

import numpy as np
import ml_dtypes

import concourse.bass as bass
import concourse.bacc as bacc
import concourse.tile as tile
from concourse import mybir
from concourse.bass_utils import run_bass_kernel_spmd

BF16 = mybir.dt.bfloat16
F32 = mybir.dt.float32
X = mybir.AxisListType.X
Exp = mybir.ActivationFunctionType.Exp
Copy = mybir.ActivationFunctionType.Copy

B, N, O, I, D = 128, 2048, 32, 8, 16
CORES = 8
BL = B // CORES            # 16 batch elements per core
J2 = N // 16               # 128 blocks of 16 input caps
OD = O * D                 # 512

_BF = ml_dtypes.bfloat16


def _bcast_last(ap, count):
    """Append a step-0 (broadcast) innermost dim to an AP."""
    return bass.AP(tensor=ap.tensor, offset=ap.offset, ap=list(ap.ap) + [[0, count]])


def build_nc():
    nc = bacc.Bacc("TRN2", target_bir_lowering=False)

    w = nc.dram_tensor("w", [J2, 128, OD], BF16, kind="ExternalInput")
    xt = nc.dram_tensor("xt", [J2, 128, BL], BF16, kind="ExternalInput")
    xbd = nc.dram_tensor("xbd", [J2, 128, 2 * 128], BF16, kind="ExternalInput")
    ones = nc.dram_tensor("ones", [128, 8], BF16, kind="ExternalInput")
    out = nc.dram_tensor("out", [BL, OD], F32, kind="ExternalOutput")

    with tile.TileContext(nc) as tc, ExitStack() as ctx:
        wp = ctx.enter_context(tc.tile_pool(name="wp", bufs=1))
        xbdp = ctx.enter_context(tc.tile_pool(name="xbdp", bufs=8))
        const = ctx.enter_context(tc.tile_pool(name="const", bufs=1))
        biasp = ctx.enter_context(tc.tile_pool(name="biasp", bufs=1))
        vexpp = ctx.enter_context(tc.tile_pool(name="vexpp", bufs=3))
        work = ctx.enter_context(tc.tile_pool(name="work", bufs=4))
        small = ctx.enter_context(tc.tile_pool(name="small", bufs=6))
        sqp = ctx.enter_context(tc.tile_pool(name="sqp", bufs=2))
        psum_u = ctx.enter_context(tc.tile_pool(name="psum_u", bufs=3, space="PSUM"))
        psum_s = ctx.enter_context(tc.tile_pool(name="psum_s", bufs=1, space="PSUM"))
        dramp = ctx.enter_context(tc.tile_pool(name="dramp", bufs=4, space="DRAM"))

        ones_sb = const.tile([128, 8], BF16)
        nc.sync.dma_start(out=ones_sb[:], in_=ones[:])
        w_all = const.tile([128, J2, OD], BF16)
        w_r = w[:].rearrange("j p f -> p j f")
        for ch in range(8):
            nc.sync.dma_start(
                out=w_all[:, ch * 16 : (ch + 1) * 16, :],
                in_=w_r[:, ch * 16 : (ch + 1) * 16, :],
            )
        xt_all = const.tile([128, J2, BL], BF16)
        nc.sync.dma_start(out=xt_all[:], in_=xt[:].rearrange("j p b -> p j b"))
        bias_h = [
            biasp.tile([128, J2 * O], F32, name="bias0"),
            biasp.tile([128, J2 * O], F32, name="bias1"),
        ]

        def squash(s_ps, P, v_ap):
            """v = s * |s|^2/(1+|s|^2) / sqrt(|s|^2 + 1e-8), per (b, o) over d."""
            s_sb = sqp.tile([P, OD], F32, tag="s_sb")
            nc.scalar.activation(s_sb[:], s_ps[:], Copy)
            ssq = sqp.tile([P, OD], F32, tag="ssq")
            nc.vector.tensor_mul(ssq[:], s_sb[:], s_sb[:])
            sq = sqp.tile([P, O], F32, tag="sq")
            nc.vector.reduce_sum(
                out=sq[:], in_=ssq[:].rearrange("p (o d) -> p o d", d=D), axis=X
            )
            d1 = sqp.tile([P, O], F32, tag="d1")
            nc.vector.tensor_scalar_add(d1[:], sq[:], 1.0)
            r1 = sqp.tile([P, O], F32, tag="r1")
            nc.vector.reciprocal(r1[:], d1[:])
            t = sqp.tile([P, O], F32, tag="t")
            nc.vector.tensor_mul(t[:], sq[:], r1[:])
            d2 = sqp.tile([P, O], F32, tag="d2")
            nc.vector.tensor_scalar_add(d2[:], sq[:], 1e-8)
            rt = sqp.tile([P, O], F32, tag="rt")
            nc.scalar.sqrt(rt[:], d2[:])
            rs = sqp.tile([P, O], F32, tag="rs")
            nc.vector.reciprocal(rs[:], rt[:])
            scale = sqp.tile([P, O], F32, tag="scale")
            nc.vector.tensor_mul(scale[:], t[:], rs[:])
            nc.vector.tensor_mul(
                v_ap.rearrange("p (o d) -> p o d", d=D),
                s_sb[:].rearrange("p (o d) -> p o d", d=D),
                _bcast_last(scale[:], D),
            )

        def make_vexp(parts):
            """parts: [(ap [rows, OD] bf16, dram row offset)] -> 2 bcast tiles."""
            vdram = dramp.tile([BL, OD], BF16, tag="vdram")
            for ap_, off in parts:
                nc.gpsimd.dma_start(out=vdram[off : off + ap_.shape[0], :], in_=ap_)
            vexp = []
            for h in range(2):
                vx = vexpp.tile([128, OD], BF16, tag=f"vexp{h}")
                for g in range(16):
                    nc.gpsimd.dma_start(
                        out=vx[g * 8 : (g + 1) * 8, :],
                        in_=vdram[h * 8 : (h + 1) * 8, :],
                    )
                vexp.append(vx)
            return vexp

        # ---------------- pass 1 (iter 0): s0 = sum_n u_hat / 32 ----------------
        s0_ps = psum_s.tile([BL, OD], F32, tag="sacc")
        for j2 in range(J2):
            nc.tensor.matmul(
                s0_ps[:],
                xt_all[:, j2, :],
                w_all[:, j2, :],
                start=(j2 == 0),
                stop=(j2 == J2 - 1),
            )
        v_bf = sqp.tile([BL, OD], BF16, tag="vbf")
        squash(s0_ps, BL, v_bf[:])
        vexp = make_vexp([(v_bf[:], 0)])

        # ---------------- passes 2, 3 (iters 1, 2) ----------------
        for k in (1, 2):
            s_ps = [
                psum_s.tile([8, OD], F32, name="sacc0", tag="sacc0"),
                psum_s.tile([8, OD], F32, name="sacc1", tag="sacc1"),
            ]
            for j2 in range(J2):
                xbd_t = xbdp.tile([128, 2 * 128], BF16)
                nc.gpsimd.dma_start(out=xbd_t[:], in_=xbd[:][j2, :, :])
                for h in range(2):
                    u_ps = psum_u.tile([128, OD], F32)
                    nc.tensor.matmul(
                        u_ps[:],
                        xbd_t[:, h * 128 : (h + 1) * 128],
                        w_all[:, j2, :],
                        start=True,
                        stop=True,
                    )
                    u_sb = work.tile([128, OD], BF16, tag="usb")
                    nc.scalar.activation(u_sb[:], u_ps[:], Copy)
                    # a[b,n,o] = sum_d u*v
                    q = work.tile([128, OD], BF16, tag="q")
                    nc.vector.tensor_mul(q[:], u_sb[:], vexp[h][:])
                    bias_slice = bias_h[h][:, j2 * O : (j2 + 1) * O]
                    if k == 1:
                        nc.vector.reduce_sum(
                            out=bias_slice,
                            in_=q[:].rearrange("p (o d) -> p o d", d=D),
                            axis=X,
                        )
                    else:
                        a2 = small.tile([128, O], F32, tag="a2")
                        nc.vector.reduce_sum(
                            out=a2[:],
                            in_=q[:].rearrange("p (o d) -> p o d", d=D),
                            axis=X,
                        )
                        nc.vector.tensor_add(bias_slice, bias_slice, a2[:])
                    # c = softmax_o(bias)
                    ex = small.tile([128, O], BF16, tag="ex")
                    sumexp = small.tile([128, 1], F32, tag="sumexp")
                    nc.scalar.activation(ex[:], bias_slice, Exp, accum_out=sumexp[:])
                    rse = small.tile([128, 1], F32, tag="rse")
                    nc.vector.reciprocal(rse[:], sumexp[:])
                    c_t = small.tile([128, O], BF16, tag="ct")
                    nc.vector.tensor_scalar_mul(c_t[:], ex[:], rse[:])
                    # e = u * c (c broadcast over d), s += ones^T @ e
                    e_t = work.tile([128, OD], BF16, tag="et")
                    nc.vector.tensor_mul(
                        e_t[:].rearrange("p (o d) -> p o d", d=D),
                        u_sb[:].rearrange("p (o d) -> p o d", d=D),
                        _bcast_last(c_t[:], D),
                    )
                    nc.tensor.matmul(
                        s_ps[h][:],
                        ones_sb[:],
                        e_t[:],
                        start=(j2 == 0),
                        stop=(j2 == J2 - 1),
                    )
            if k == 1:
                vtmps = []
                for h in range(2):
                    vtmp = sqp.tile([8, OD], BF16, tag="vtmp")
                    squash(s_ps[h], 8, vtmp[:])
                    vtmps.append(vtmp)
                vexp = make_vexp([(vtmps[0][:], 0), (vtmps[1][:], 8)])
            else:
                for h in range(2):
                    v_f32 = sqp.tile([8, OD], F32, tag="vf32")
                    squash(s_ps[h], 8, v_f32[:])
                    nc.gpsimd.dma_start(out=out[:][h * 8 : (h + 1) * 8, :], in_=v_f32[:])

    nc.compile()
    return nc


_nc_cache = {}


def _get_nc():
    if "nc" not in _nc_cache:
        _nc_cache["nc"] = build_nc()
    return _nc_cache["nc"]


def _prep_host(x, W):
    """Build the per-core input maps (numpy only)."""
    # W16[j2][(n,i)][(o,d)] = W[16*j2+n, o, i, d]
    W16 = (
        W.reshape(J2, 16, O, I, D)
        .transpose(0, 1, 3, 2, 4)
        .reshape(J2, 128, OD)
        .astype(_BF)
    )
    ones_bd = np.zeros((128, 8), dtype=_BF)
    for p in range(128):
        ones_bd[p, p % 8] = 1.0
    in_maps = []
    for c in range(CORES):
        xl = x[c * BL : (c + 1) * BL]  # [16, 2048, 8]
        T = xl.reshape(BL, J2, 16, I).transpose(1, 2, 3, 0)  # [j2, n, i, b]
        xt = (T[:, :, :, :] / 32.0).reshape(J2, 128, BL).astype(_BF)
        xbd = np.zeros((J2, 128, 2, 128), dtype=np.float32)
        for n in range(16):
            xbd[:, n * 8 : (n + 1) * 8, 0, n * 8 : (n + 1) * 8] = T[:, n, :, 0:8]
            xbd[:, n * 8 : (n + 1) * 8, 1, n * 8 : (n + 1) * 8] = T[:, n, :, 8:16]
        in_maps.append(
            {
                "w": W16,
                "xt": xt,
                "xbd": xbd.reshape(J2, 128, 256).astype(_BF),
                "ones": ones_bd,
            }
        )
    return in_maps


TRACE = False
_last = {}


def kernel(x: np.ndarray, W: np.ndarray) -> np.ndarray:
    nc = _get_nc()
    in_maps = _prep_host(
        np.asarray(x, dtype=np.float32), np.asarray(W, dtype=np.float32)
    )
    res = run_bass_kernel_spmd(
        nc, in_maps, core_ids=list(range(CORES)), trace=TRACE
    )
    _last["res"] = res
    outs = [r["out"].reshape(BL, O, D) for r in res.results]
    return np.concatenate(outs, axis=0).astype(np.float32)


if __name__ == "__main__":
    rng = np.random.default_rng(0)
    x = rng.standard_normal((B, N, I), dtype=np.float32)
    W = rng.standard_normal((N, O, I, D), dtype=np.float32)
    v = kernel(x, W)
    print(v.shape, v.dtype, float(np.abs(v).mean()))


# revision 37
# speedup vs baseline: 120.4592x; 120.4592x over previous
"""CapsuleLayer (dynamic routing) Trainium2 kernel.

x: [128, 2048, 8] f32, W: [2048, 32, 8, 16] f32 -> v: [128, 32, 16] f32

Sharding: batch B=128 split across 8 cores (16 each), W replicated
(resident in SBUF, bf16). Per core, per routing pass, u_hat tiles
([128, 1024] = 16 caps x 16 batch x 512 (o,d)) are recomputed on the PE
via a block-diagonal-x matmul and consumed on-chip; u_hat never touches
HBM. Engine split: PE produce + softmax-weighted n-reduction (matmul
with block-ones lhsT), ACT PSUM drain + exp, DVE multiplies, Pool
d-reduce.
"""

from contextlib import ExitStack

import numpy as np
import ml_dtypes

import concourse.bass as bass
import concourse.bacc as bacc
import concourse.tile as tile
from concourse import mybir
from concourse.bass_utils import run_bass_kernel_spmd

BF16 = mybir.dt.bfloat16
F32 = mybir.dt.float32
X = mybir.AxisListType.X
Exp = mybir.ActivationFunctionType.Exp
Copy = mybir.ActivationFunctionType.Copy

B, N, O, I, D = 128, 2048, 32, 8, 16
CORES = 8
BL = B // CORES            # 16 batch elements per core
J2 = N // 16               # 128 blocks of 16 input caps
OD = O * D                 # 512
G = 4                      # j2 group size for batched softmax

_BF = ml_dtypes.bfloat16


def _bcast_last(ap, count):
    """Append a step-0 (broadcast) innermost dim to an AP."""
    return bass.AP(tensor=ap.tensor, offset=ap.offset, ap=list(ap.ap) + [[0, count]])


def build_nc():
    nc = bacc.Bacc("TRN2", target_bir_lowering=False)

    w = nc.dram_tensor("w", [J2, 128, OD], BF16, kind="ExternalInput")
    xt = nc.dram_tensor("xt", [J2, 128, BL], BF16, kind="ExternalInput")
    xbd = nc.dram_tensor("xbd", [J2, 128, 2 * 128], BF16, kind="ExternalInput")
    ones = nc.dram_tensor("ones", [128, 8], BF16, kind="ExternalInput")
    out = nc.dram_tensor("out", [BL, OD], F32, kind="ExternalOutput")

    with tile.TileContext(nc) as tc, ExitStack() as ctx:
        xbdp = ctx.enter_context(tc.tile_pool(name="xbdp", bufs=4))
        const = ctx.enter_context(tc.tile_pool(name="const", bufs=1))
        biasp = ctx.enter_context(tc.tile_pool(name="biasp", bufs=1))
        vexpp = ctx.enter_context(tc.tile_pool(name="vexpp", bufs=2))
        work = ctx.enter_context(tc.tile_pool(name="work", bufs=3))
        small = ctx.enter_context(tc.tile_pool(name="small", bufs=4))
        sqp = ctx.enter_context(tc.tile_pool(name="sqp", bufs=1))
        psum_u = ctx.enter_context(tc.tile_pool(name="psum_u", bufs=2, space="PSUM"))
        psum_s = ctx.enter_context(tc.tile_pool(name="psum_s", bufs=1, space="PSUM"))
        dramp = ctx.enter_context(tc.tile_pool(name="dramp", bufs=4, space="DRAM"))

        ones_sb = const.tile([128, 8], BF16)
        nc.sync.dma_start(out=ones_sb[:], in_=ones[:])
        xt_all = const.tile([128, J2, BL], BF16)
        nc.sync.dma_start(out=xt_all[:], in_=xt[:].rearrange("j p b -> p j b"))
        w_all = const.tile([128, J2, OD], BF16)
        w_r = w[:].rearrange("j p f -> p j f")
        for ch in range(8):
            nc.sync.dma_start(
                out=w_all[:, ch * 16 : (ch + 1) * 16, :],
                in_=w_r[:, ch * 16 : (ch + 1) * 16, :],
            )
        # bias[(n16 b8) partition, (j2, h, o)] f32
        bias_all = biasp.tile([128, J2, 2, O], F32)

        def squash(s_ap, P, v_ap):
            """v = s * |s|^2/(1+|s|^2) / sqrt(|s|^2 + 1e-8), per (b, o) over d."""
            s_sb = sqp.tile([P, OD], F32, tag="s_sb")
            nc.scalar.activation(s_sb[:], s_ap, Copy)
            ssq = sqp.tile([P, OD], F32, tag="ssq")
            nc.vector.tensor_mul(ssq[:], s_sb[:], s_sb[:])
            sq = sqp.tile([P, O], F32, tag="sq")
            nc.vector.reduce_sum(
                out=sq[:], in_=ssq[:].rearrange("p (o d) -> p o d", d=D), axis=X
            )
            d1 = sqp.tile([P, O], F32, tag="d1")
            nc.vector.tensor_scalar_add(d1[:], sq[:], 1.0)
            r1 = sqp.tile([P, O], F32, tag="r1")
            nc.vector.reciprocal(r1[:], d1[:])
            t = sqp.tile([P, O], F32, tag="t")
            nc.vector.tensor_mul(t[:], sq[:], r1[:])
            d2 = sqp.tile([P, O], F32, tag="d2")
            nc.vector.tensor_scalar_add(d2[:], sq[:], 1e-8)
            rt = sqp.tile([P, O], F32, tag="rt")
            nc.scalar.sqrt(rt[:], d2[:])
            rs = sqp.tile([P, O], F32, tag="rs")
            nc.vector.reciprocal(rs[:], rt[:])
            scale = sqp.tile([P, O], F32, tag="scale")
            nc.vector.tensor_mul(scale[:], t[:], rs[:])
            nc.vector.tensor_mul(
                v_ap.rearrange("p (o d) -> p o d", d=D),
                s_sb[:].rearrange("p (o d) -> p o d", d=D),
                _bcast_last(scale[:], D),
            )

        def make_vexp(parts):
            """parts: [(ap [rows, OD] bf16, dram row offset)] -> [128, 2*OD] tile.

            vexp[p=(n16 b8), h*OD + (o,d)] = v[h*8 + p%8, o, d]
            """
            vdram = dramp.tile([BL, OD], BF16, tag="vdram")
            for ap_, off in parts:
                nc.sync.dma_start(out=vdram[off : off + ap_.shape[0], :], in_=ap_)
            vx = vexpp.tile([128, 2 * OD], BF16, tag="vexp")
            for h in range(2):
                for g in range(16):
                    nc.sync.dma_start(
                        out=vx[g * 8 : (g + 1) * 8, h * OD : (h + 1) * OD],
                        in_=vdram[h * 8 : (h + 1) * 8, :],
                    )
            return vx

        # ---------------- pass 1 (iter 0): s0 = sum_n u_hat / 32 ----------------
        s0_ps = psum_s.tile([BL, OD], F32, tag="sacc0")
        for j2 in range(J2):
            nc.tensor.matmul(
                s0_ps[:],
                xt_all[:, j2, :],
                w_all[:, j2, :],
                start=(j2 == 0),
                stop=(j2 == J2 - 1),
            )
        v_bf = sqp.tile([BL, OD], BF16, tag="vbf")
        squash(s0_ps[:], BL, v_bf[:])
        vexp = make_vexp([(v_bf[:], 0)])

        # ---------------- passes 2, 3 (iters 1, 2) ----------------
        for k in (1, 2):
            # s accumulator [8, 2*OD]: cols h*OD+(o,d) for batch b = h*8 + row
            s_ps = psum_s.tile([8, 2 * OD], F32, name=f"sacc_{k}", tag="sacc1")
            for jg in range(J2 // G):
                usb_g = []
                for jj in range(G):
                    j2 = jg * G + jj
                    xbd_t = xbdp.tile([128, 2 * 128], BF16)
                    nc.sync.dma_start(out=xbd_t[:], in_=xbd[:][j2, :, :])
                    u_ps = psum_u.tile([128, 2 * OD], F32)
                    for h in range(2):
                        nc.tensor.matmul(
                            u_ps[:, h * OD : (h + 1) * OD],
                            xbd_t[:, h * 128 : (h + 1) * 128],
                            w_all[:, j2, :],
                            start=True,
                            stop=True,
                        )
                    u_sb = work.tile([128, 2 * OD], BF16, tag="usb", bufs=5)
                    nc.scalar.activation(u_sb[:], u_ps[:], Copy)
                    usb_g.append(u_sb)
                    # a[b,n,(h,o)] = sum_d u*v  (both halves in one op)
                    q = work.tile([128, 2 * OD], BF16, tag="q", bufs=3)
                    nc.vector.tensor_mul(q[:], u_sb[:], vexp[:])
                    bias_slice = bias_all[:, j2, :, :]  # [128, 2, O]
                    a_out = (
                        bias_slice
                        if k == 1
                        else small.tile([128, 2, O], F32, name="a2", tag="a2")[:]
                    )
                    nc.vector.reduce_sum(
                        out=a_out,
                        in_=q[:].rearrange("p (h o d) -> p h o d", h=2, d=D),
                        axis=X,
                    )
                    if k != 1:
                        nc.vector.tensor_add(bias_slice, bias_slice, a_out)
                # grouped softmax over the G j2's just processed
                bias_g = bias_all[:, jg * G : (jg + 1) * G, :, :]  # [128,G,2,O]
                ex = small.tile([128, G, 2, O], BF16, tag="ex")
                nc.scalar.activation(ex[:], bias_g, Exp)
                se = small.tile([128, G, 2], F32, tag="se")
                nc.vector.reduce_sum(out=se[:], in_=ex[:], axis=X)
                rse = small.tile([128, G, 2], F32, tag="rse")
                nc.vector.reciprocal(rse[:], se[:])
                c_t = small.tile([128, G, 2, O], BF16, tag="ct")
                nc.vector.tensor_mul(c_t[:], ex[:], _bcast_last(rse[:], O))
                for jj in range(G):
                    j2 = jg * G + jj
                    # e = u * c (c broadcast over d), s += ones^T @ e
                    e_t = work.tile([128, 2 * OD], BF16, tag="et", bufs=3)
                    nc.vector.tensor_mul(
                        e_t[:].rearrange("p (h o d) -> p h o d", h=2, d=D),
                        usb_g[jj][:].rearrange("p (h o d) -> p h o d", h=2, d=D),
                        _bcast_last(c_t[:, jj, :, :], D),
                    )
                    for h in range(2):
                        nc.tensor.matmul(
                            s_ps[:, h * OD : (h + 1) * OD],
                            ones_sb[:],
                            e_t[:, h * OD : (h + 1) * OD],
                            start=(j2 == 0),
                            stop=(j2 == J2 - 1),
                        )
            if k == 1:
                vtmps = []
                for h in range(2):
                    vtmp = sqp.tile([8, OD], BF16, tag="vtmp")
                    squash(s_ps[:, h * OD : (h + 1) * OD], 8, vtmp[:])
                    vtmps.append(vtmp)
                vexp = make_vexp([(vtmps[0][:], 0), (vtmps[1][:], 8)])
            else:
                for h in range(2):
                    v_f32 = sqp.tile([8, OD], F32, tag="vf32")
                    squash(s_ps[:, h * OD : (h + 1) * OD], 8, v_f32[:])
                    nc.sync.dma_start(
                        out=out[:][h * 8 : (h + 1) * 8, :], in_=v_f32[:]
                    )

    nc.compile()
    return nc


_nc_cache = {}


def _get_nc():
    if "nc" not in _nc_cache:
        _nc_cache["nc"] = build_nc()
    return _nc_cache["nc"]


def _prep_host(x, W):
    """Build the per-core input maps (numpy only)."""
    # W16[j2][(n,i)][(o,d)] = W[16*j2+n, o, i, d]
    W16 = (
        W.reshape(J2, 16, O, I, D)
        .transpose(0, 1, 3, 2, 4)
        .reshape(J2, 128, OD)
        .astype(_BF)
    )
    ones_bd = np.zeros((128, 8), dtype=_BF)
    for p in range(128):
        ones_bd[p, p % 8] = 1.0
    in_maps = []
    for c in range(CORES):
        xl = x[c * BL : (c + 1) * BL]  # [16, 2048, 8]
        T = xl.reshape(BL, J2, 16, I).transpose(1, 2, 3, 0)  # [j2, n, i, b]
        xt = (T / 32.0).reshape(J2, 128, BL).astype(_BF)
        xbd = np.zeros((J2, 128, 2, 128), dtype=np.float32)
        for n in range(16):
            xbd[:, n * 8 : (n + 1) * 8, 0, n * 8 : (n + 1) * 8] = T[:, n, :, 0:8]
            xbd[:, n * 8 : (n + 1) * 8, 1, n * 8 : (n + 1) * 8] = T[:, n, :, 8:16]
        in_maps.append(
            {
                "w": W16,
                "xt": xt,
                "xbd": xbd.reshape(J2, 128, 256).astype(_BF),
                "ones": ones_bd,
            }
        )
    return in_maps


TRACE = False
_last = {}


def kernel(x: np.ndarray, W: np.ndarray) -> np.ndarray:
    nc = _get_nc()
    in_maps = _prep_host(
        np.asarray(x, dtype=np.float32), np.asarray(W, dtype=np.float32)
    )
    res = run_bass_kernel_spmd(
        nc, in_maps, core_ids=list(range(CORES)), trace=TRACE
    )
    _last["res"] = res
    outs = [r["out"].reshape(BL, O, D) for r in res.results]
    return np.concatenate(outs, axis=0).astype(np.float32)


if __name__ == "__main__":
    rng = np.random.default_rng(0)
    x = rng.standard_normal((B, N, I), dtype=np.float32)
    W = rng.standard_normal((N, O, I, D), dtype=np.float32)
    v = kernel(x, W)
    print(v.shape, v.dtype, float(np.abs(v).mean()))
